# revision 26
# baseline (speedup 1.0000x reference)
"""Trainium2 Bass kernel for nn_BCELoss_64330020159675 (segment_reduce BCE loss).

Data-parallel over batch across 8 NeuronCores:
  phase A (per core, local batch shard of 1024 rows):
    z_i = normalize(emb_i); onehot = (labels == arange(C));
    segT[d, c] = sum_b z_i[b, d] * onehot[b, c]  (PE matmuls, d-major output)
    counts[c] = sum_b onehot[b, c]               (PE matmuls, c-major columns)
  AllReduce of [D+1, C] f32 (segT rows 0..D-1, counts packed in row D).
  phase B (overlaps the collective): load emb_j pre-transposed [D, B_local],
    column norms via Square + partition_all_reduce, z_jT = emb_jT * invnorm.
  phase C: Q[c, b] = sum_d segT[d, c] * z_jT[d, b] (PE matmuls);
    d2 = Q * (-2/cnt_c) + (1 + |seg_c|^2/cnt_c^2)  -> folded into the Sqrt
    activation's per-partition scale/bias; r = sqrt(d2); sim = 2 - r;
    softplus(sim) = Ln(Exp(-r + 2) + 1) with per-row accumulation;
    diag term sum_b r[b, label_b] via onehot * r reduce.
  Host: loss = (sum_cores(sp_total + diag_total) - 2B) / (B*C).

Identity used: BCEWithLogits elementwise loss = softplus(sim) - match * sim,
and sum(match * sim) = 2*B - sum_b r[b, label_b].
"""
import numpy as np

import concourse.bass as bass
import concourse.bacc as bacc
import concourse.mybir as mybir
import concourse.tile as tile
from concourse import bass_utils

B = 8192
D = 1024
C = 1024
N_CORES = 8
BL = B // N_CORES          # 1024 rows per core
P = 128                    # partitions
NB = BL // P               # 8 batch chunks per core
ND = D // P                # 8 d chunks
NCC = C // P               # 8 class chunks (partition-major)
NBF = BL // 512            # 2 batch free-dim chunks
NCF = C // 512             # 2 class free-dim chunks
NBLK = NCC * NBF           # 16 sim blocks
EPS = 1e-12

F32 = mybir.dt.float32
BF16 = mybir.dt.bfloat16
AF = mybir.ActivationFunctionType
ALU = mybir.AluOpType
AX = mybir.AxisListType

_NC_CACHE = {}


def build_nc():
    if "nc" in _NC_CACHE:
        return _NC_CACHE["nc"]
    import concourse.bass_isa as bass_isa

    nc = bacc.Bacc(
        "TRN2", target_bir_lowering=False, debug=False, num_devices=N_CORES
    )
    emb_i = nc.dram_tensor("emb_i", [BL, D], F32, kind="ExternalInput")
    emb_jT = nc.dram_tensor("emb_jT", [D, BL], F32, kind="ExternalInput")
    labels_colmat = nc.dram_tensor("labels_colmat", [P, NB], F32, kind="ExternalInput")
    label_bcast = nc.dram_tensor("label_bcast", [P, BL], F32, kind="ExternalInput")
    iota_bcast = nc.dram_tensor("iota_bcast", [P, C], F32, kind="ExternalInput")
    ccol = nc.dram_tensor("ccol", [P, NCC], F32, kind="ExternalInput")
    out_partial = nc.dram_tensor("out_partial", [1, 2], F32, kind="ExternalOutput")

    with tile.TileContext(nc) as tc:
        with (
            tc.tile_pool(name="dram", bufs=1, space="DRAM") as dram,
            tc.tile_pool(name="const", bufs=1) as constp,
            tc.tile_pool(name="zjt", bufs=1) as zjtp,
            tc.tile_pool(name="work", bufs=2) as work,
            tc.tile_pool(name="work3", bufs=3) as work3,
            tc.tile_pool(name="dump", bufs=1) as dump,
        ):
            HD = D // 2  # collective split point (rows 0..HD-1 | HD..D + counts)
            cc_in0 = dram.tile([HD, C], BF16)
            cc_out0 = dram.tile([HD, C], BF16, addr_space="Shared")
            cc_in1 = dram.tile([HD + 1, C], BF16)
            cc_out1 = dram.tile([HD + 1, C], BF16, addr_space="Shared")

            ones_col = constp.tile([P, 1], F32)
            nc.vector.memset(ones_col[:], 1.0)
            ones_bf = constp.tile([P, 1], BF16)
            nc.vector.memset(ones_bf[:], 1.0)
            two_col = constp.tile([P, 1], F32)
            nc.vector.memset(two_col[:], 2.0)
            lab_cm = constp.tile([P, NB], F32)
            nc.sync.dma_start(lab_cm[:], labels_colmat[:])
            lab_bc = constp.tile([P, BL], F32)
            nc.sync.dma_start(lab_bc[:], label_bcast[:])
            iota_bc = constp.tile([P, C], F32)
            nc.sync.dma_start(iota_bc[:], iota_bcast[:])
            ccol_t = constp.tile([P, NCC], F32)
            nc.sync.dma_start(ccol_t[:], ccol[:])

            # ---------------- phase A ----------------
            with (
                tc.tile_pool(name="phA", bufs=1) as pa,
                tc.tile_pool(name="psA", bufs=2, space="PSUM") as psA,
            ):
                z_i = [pa.tile([P, D], BF16, name=f"zi{b}") for b in range(NB)]
                oh = [pa.tile([P, C], BF16, name=f"oh{b}") for b in range(NB)]
                sq_dump = dump.tile([P, D], F32, name="sq_dump")
                # per-chunk pipelined norms; Square/Sqrt share one ACT table
                for b in range(NB):
                    e = work.tile([P, D], F32, tag="embi")
                    nc.sync.dma_start(e[:], emb_i[b * P : (b + 1) * P, :])
                    ss = work.tile([P, 1], F32, tag="ss")
                    nc.scalar.activation(sq_dump[:], e[:], AF.Square, accum_out=ss[:])
                    nrm = work.tile([P, 1], F32, tag="nrm")
                    nc.scalar.activation(nrm[:], ss[:], AF.Sqrt)
                    nc.vector.tensor_scalar(nrm[:], nrm[:], EPS, None, ALU.max)
                    inv = work.tile([P, 1], F32, tag="inv")
                    nc.vector.reciprocal(inv[:], nrm[:])
                    nc.vector.tensor_scalar(z_i[b][:], e[:], inv[:], None, ALU.mult)
                    nc.vector.tensor_scalar(
                        oh[b][:], iota_bc[:], lab_cm[:, b : b + 1], None, ALU.is_equal
                    )

                # segT matmuls: out[d_chunk, c] = sum_b z_i[b, d] * onehot[b, c]
                # d 0..3 feed cc_in0 (collective chunk 0, launched early);
                # d 4..7 + counts feed cc_in1 (collective chunk 1).
                for d in range(ND):
                    for cf in range(NCF):
                        ps = psA.tile([P, 512], F32, tag="seg")
                        for b in range(NB):
                            nc.tensor.matmul(
                                ps[:],
                                z_i[b][:, d * P : (d + 1) * P],
                                oh[b][:, cf * 512 : (cf + 1) * 512],
                                start=(b == 0),
                                stop=(b == NB - 1),
                            )
                        so = work3.tile([P, 512], BF16, tag="segout")
                        nc.vector.tensor_copy(so[:], ps[:])
                        cc = cc_in0 if d < ND // 2 else cc_in1
                        row = (d % (ND // 2)) * P
                        nc.sync.dma_start(
                            cc[row : row + P, cf * 512 : (cf + 1) * 512], so[:]
                        )
                    if d == ND // 2 - 1:
                        nc.gpsimd.collective_compute(
                            "AllReduce",
                            ALU.add,
                            replica_groups=[list(range(N_CORES))],
                            ins=[cc_in0[:].opt()],
                            outs=[cc_out0[:].opt()],
                        )

                # counts as a [1, C] row: cnt[c] = sum_b onehot[b, c]
                cnt_ps = psA.tile([1, C], F32, tag="cntrow")
                for half in range(NCF):
                    for b in range(NB):
                        nc.tensor.matmul(
                            cnt_ps[:, half * 512 : (half + 1) * 512],
                            ones_bf[:],
                            oh[b][:, half * 512 : (half + 1) * 512],
                            start=(b == 0),
                            stop=(b == NB - 1),
                        )
                cnt_row = work.tile([1, C], BF16, tag="cntrowsb")
                nc.vector.tensor_copy(cnt_row[:], cnt_ps[:])
                nc.sync.dma_start(cc_in1[HD : HD + 1, :], cnt_row[:])

            nc.gpsimd.collective_compute(
                "AllReduce",
                ALU.add,
                replica_groups=[list(range(N_CORES))],
                ins=[cc_in1[:].opt()],
                outs=[cc_out1[:].opt()],
            )

            # ---------------- phase B (overlaps collective) ----------------
            zjt = [zjtp.tile([P, BL], BF16, name=f"zjt{d}") for d in range(ND)]
            with tc.tile_pool(name="embt", bufs=1) as embtp:
                embT = [embtp.tile([P, BL], F32, name=f"embT{d}") for d in range(ND)]
                acc = embtp.tile([P, BL], F32, name="acc")
                for d in range(ND):
                    nc.sync.dma_start(embT[d][:], emb_jT[d * P : (d + 1) * P, :])
                    sq2 = work.tile([P, BL], F32, tag="sqscr2")
                    nc.scalar.activation(sq2[:], embT[d][:], AF.Square)
                    if d == 0:
                        nc.vector.tensor_copy(acc[:], sq2[:])
                    else:
                        nc.vector.tensor_add(acc[:], acc[:], sq2[:])
                nrm2 = embtp.tile([P, BL], F32, name="nrm2")
                nc.gpsimd.partition_all_reduce(
                    nrm2[:], acc[:], channels=P, reduce_op=bass_isa.ReduceOp.add
                )
                nc.scalar.activation(nrm2[:], nrm2[:], AF.Sqrt)
                nc.vector.tensor_scalar(nrm2[:], nrm2[:], EPS, None, ALU.max)
                invb = embtp.tile([P, BL], F32, name="invb")
                nc.vector.reciprocal(invb[:], nrm2[:])
                for d in range(ND):
                    nc.vector.tensor_tensor(zjt[d][:], embT[d][:], invb[:], ALU.mult)

            # ---------------- phase C ----------------
            with (
                tc.tile_pool(name="phC", bufs=1) as pcpool,
                tc.tile_pool(name="psC", bufs=2, space="PSUM") as psC,
                tc.tile_pool(name="psSim", bufs=3, space="PSUM") as psSim,
            ):
                segT = [pcpool.tile([P, C], BF16, name=f"segT{d}") for d in range(ND)]
                # ssq accumulated on DVE as segT chunks arrive
                ssqacc = pcpool.tile([P, C], F32, name="ssqacc")
                for d in range(ND):
                    src = cc_out0 if d < ND // 2 else cc_out1
                    row = (d % (ND // 2)) * P
                    nc.sync.dma_start(segT[d][:], src[row : row + P, :])
                    sqc = work.tile([P, C], F32, tag="sqc")
                    nc.scalar.activation(sqc[:], segT[d][:], AF.Square)
                    if d == 0:
                        nc.vector.tensor_copy(ssqacc[:], sqc[:])
                    else:
                        nc.vector.tensor_add(ssqacc[:], ssqacc[:], sqc[:])
                ssqred = pcpool.tile([P, C], F32, name="ssqred")
                nc.gpsimd.partition_all_reduce(
                    ssqred[:], ssqacc[:], channels=P, reduce_op=bass_isa.ReduceOp.add
                )
                cnt_rowb = constp.tile([1, C], BF16)
                nc.sync.dma_start(cnt_rowb[:], cc_out1[HD : HD + 1, :])
                cnt_row2 = constp.tile([1, C], F32)
                nc.vector.tensor_copy(cnt_row2[:], cnt_rowb[:])
                ident1 = constp.tile([1, 1], F32)
                nc.vector.memset(ident1[:], 1.0)
                cnt_col = constp.tile([P, NCC], F32)
                ssq_col = constp.tile([P, NCC], F32)
                for cc in range(NCC):
                    pt = psC.tile([P, 1], F32, tag="col1")
                    nc.tensor.transpose(
                        pt[:], cnt_row2[0:1, cc * P : (cc + 1) * P], ident1[:]
                    )
                    nc.vector.tensor_copy(cnt_col[:, cc : cc + 1], pt[:])
                    pq = psC.tile([P, 1], F32, tag="col1")
                    nc.tensor.transpose(
                        pq[:], ssqred[0:1, cc * P : (cc + 1) * P], ident1[:]
                    )
                    nc.vector.tensor_copy(ssq_col[:, cc : cc + 1], pq[:])
                ic = constp.tile([P, NCC], F32)
                nc.vector.reciprocal(ic[:], cnt_col[:])
                scale_col = constp.tile([P, NCC], F32)
                nc.vector.tensor_scalar(scale_col[:], ic[:], -2.0, None, ALU.mult)
                ic2 = constp.tile([P, NCC], F32)
                nc.vector.tensor_tensor(ic2[:], ic[:], ic[:], ALU.mult)
                bias_col = constp.tile([P, NCC], F32)
                nc.vector.tensor_tensor(bias_col[:], ssq_col[:], ic2[:], ALU.mult)
                nc.vector.tensor_scalar(bias_col[:], bias_col[:], 1.0, None, ALU.add)

                sp_st = constp.tile([P, NBLK], F32)
                dg_st = constp.tile([P, NBLK], F32)
                sp_dump = dump.tile([P, 512], F32, name="sp_dump")
                with tc.tile_pool(name="rall", bufs=1) as rallp:
                    r_all = [
                        rallp.tile([P, 512], F32, name=f"r{blk}")
                        for blk in range(NBLK)
                    ]
                    # pass 1: matmuls + Sqrt per block (single ACT table),
                    # DVE diag term per block
                    for cc in range(NCC):
                        for bf in range(NBF):
                            blk = cc * NBF + bf
                            ps = psSim.tile([P, 512], F32, tag="sim")
                            for d in range(ND):
                                nc.tensor.matmul(
                                    ps[:],
                                    segT[d][:, cc * P : (cc + 1) * P],
                                    zjt[d][:, bf * 512 : (bf + 1) * 512],
                                    start=(d == 0),
                                    stop=(d == ND - 1),
                                )
                            nc.scalar.activation(
                                r_all[blk][:],
                                ps[:],
                                AF.Sqrt,
                                bias=bias_col[:, cc : cc + 1],
                                scale=scale_col[:, cc : cc + 1],
                            )
                            oht = work.tile([P, 512], F32, tag="oht")
                            nc.vector.tensor_scalar(
                                oht[:],
                                lab_bc[:, bf * 512 : (bf + 1) * 512],
                                ccol_t[:, cc : cc + 1],
                                None,
                                ALU.is_equal,
                            )
                            prod = work.tile([P, 512], F32, tag="prod")
                            nc.vector.tensor_tensor(
                                prod[:], r_all[blk][:], oht[:], ALU.mult
                            )
                            nc.vector.tensor_reduce(
                                dg_st[:, blk : blk + 1],
                                prod[:],
                                axis=AX.X,
                                op=ALU.add,
                            )
                    # pass 2: Exp + Ln batched (one table switch total).
                    # two_gate copies two_col after the last Sqrt so every Exp
                    # data-depends on all Sqrts -> scheduler cannot interleave
                    # Exp/Ln into the Sqrt stretch (would thrash ACT tables).
                    two_gate = constp.tile([P, 1], F32)
                    gate_probe = work.tile([P, 1], F32, tag="gateprobe")
                    nc.vector.tensor_reduce(
                        gate_probe[:], r_all[NBLK - 1][:, 0:2], axis=AX.X, op=ALU.max
                    )
                    nc.vector.tensor_scalar(
                        two_gate[:], gate_probe[:], 0.0, 2.0, ALU.mult, ALU.add
                    )
                    ex_all = [
                        rallp.tile([P, 512], F32, name=f"ex{blk}")
                        for blk in range(NBLK)
                    ]
                    for blk in range(NBLK):
                        nc.scalar.activation(
                            ex_all[blk][:],
                            r_all[blk][:],
                            AF.Exp,
                            bias=two_gate[:],
                            scale=-1.0,
                        )
                    one_gate = constp.tile([P, 1], F32)
                    gate_probe2 = work.tile([P, 1], F32, tag="gateprobe")
                    nc.vector.tensor_reduce(
                        gate_probe2[:],
                        ex_all[NBLK - 1][:, 0:2],
                        axis=AX.X,
                        op=ALU.max,
                    )
                    nc.vector.tensor_scalar(
                        one_gate[:], gate_probe2[:], 0.0, 1.0, ALU.mult, ALU.add
                    )
                    for blk in range(NBLK):
                        nc.scalar.activation(
                            sp_dump[:],
                            ex_all[blk][:],
                            AF.Ln,
                            bias=one_gate[:],
                            accum_out=sp_st[:, blk : blk + 1],
                        )

                # final on-device reduction to two scalars
                pf = psC.tile([1, NBLK], F32, tag="fin")
                nc.tensor.matmul(pf[:], ones_col[:], sp_st[:], start=True, stop=True)
                sp_row = constp.tile([1, NBLK], F32)
                nc.vector.tensor_copy(sp_row[:], pf[:])
                sp_tot = constp.tile([1, 1], F32)
                nc.vector.tensor_reduce(sp_tot[:], sp_row[:], axis=AX.X, op=ALU.add)

                pf2 = psC.tile([1, NBLK], F32, tag="fin")
                nc.tensor.matmul(pf2[:], ones_col[:], dg_st[:], start=True, stop=True)
                dg_row = constp.tile([1, NBLK], F32)
                nc.vector.tensor_copy(dg_row[:], pf2[:])
                dg_tot = constp.tile([1, 1], F32)
                nc.vector.tensor_reduce(dg_tot[:], dg_row[:], axis=AX.X, op=ALU.add)

                nc.sync.dma_start(out_partial[0:1, 0:1], sp_tot[:])
                nc.sync.dma_start(out_partial[0:1, 1:2], dg_tot[:])

    nc.compile()
    _NC_CACHE["nc"] = nc
    return nc


def make_in_maps(emb_i, emb_j, labels):
    emb_i = np.ascontiguousarray(np.asarray(emb_i, dtype=np.float32))
    emb_j = np.ascontiguousarray(np.asarray(emb_j, dtype=np.float32))
    labf = np.asarray(labels).astype(np.float32)
    iota_bc = np.ascontiguousarray(
        np.broadcast_to(np.arange(C, dtype=np.float32)[None, :], (P, C))
    )
    ccol = np.ascontiguousarray(
        np.arange(P, dtype=np.float32)[:, None]
        + P * np.arange(NCC, dtype=np.float32)[None, :]
    )
    in_maps = []
    for k in range(N_CORES):
        sl = slice(k * BL, (k + 1) * BL)
        lab_k = labf[sl]
        in_maps.append(
            {
                "emb_i": emb_i[sl],
                "emb_jT": np.ascontiguousarray(emb_j[sl].T),
                "labels_colmat": np.ascontiguousarray(lab_k.reshape(NB, P).T),
                "label_bcast": np.ascontiguousarray(
                    np.broadcast_to(lab_k[None, :], (P, BL))
                ),
                "iota_bcast": iota_bc,
                "ccol": ccol,
            }
        )
    return in_maps


def combine_partials(results):
    tot = 0.0
    for k in range(N_CORES):
        p = np.asarray(results[k]["out_partial"], dtype=np.float64)
        tot += p[0, 0] + p[0, 1]
    loss = (tot - 2.0 * B) / (B * C)
    return np.asarray(np.float32(loss))


def run(emb_i, emb_j, labels, **run_kwargs):
    nc = build_nc()
    in_maps = make_in_maps(emb_i, emb_j, labels)
    res = bass_utils.run_bass_kernel_spmd(
        nc, in_maps, core_ids=list(range(N_CORES)), **run_kwargs
    )
    return combine_partials(res.results), res


def kernel(emb_i, emb_j, labels):
    loss, _ = run(emb_i, emb_j, labels)
    return loss


# revision 39
# speedup vs baseline: 1.2073x; 1.2073x over previous
"""Trainium2 Bass kernel for nn_BCELoss_64330020159675 (segment_reduce BCE loss).

Data-parallel over batch across 8 NeuronCores:
  phase A (per core, local batch shard of 1024 rows):
    z_i = normalize(emb_i); onehot = (labels == arange(C));
    segT[d, c] = sum_b z_i[b, d] * onehot[b, c]  (PE matmuls, d-major output)
    counts[c] = sum_b onehot[b, c]               (PE matmuls, c-major columns)
  AllReduce of [D+1, C] f32 (segT rows 0..D-1, counts packed in row D).
  phase B (overlaps the collective): load emb_j pre-transposed [D, B_local],
    column norms via Square + partition_all_reduce, z_jT = emb_jT * invnorm.
  phase C: Q[c, b] = sum_d segT[d, c] * z_jT[d, b] (PE matmuls);
    d2 = Q * (-2/cnt_c) + (1 + |seg_c|^2/cnt_c^2)  -> folded into the Sqrt
    activation's per-partition scale/bias; r = sqrt(d2); sim = 2 - r;
    softplus(sim) = Ln(Exp(-r + 2) + 1) with per-row accumulation;
    diag term sum_b r[b, label_b] via onehot * r reduce.
  Host: loss = (sum_cores(sp_total + diag_total) - 2B) / (B*C).

Identity used: BCEWithLogits elementwise loss = softplus(sim) - match * sim,
and sum(match * sim) = 2*B - sum_b r[b, label_b].
"""
import numpy as np

import concourse.bass as bass
import concourse.bacc as bacc
import concourse.mybir as mybir
import concourse.tile as tile
from concourse import bass_utils

B = 8192
D = 1024
C = 1024
N_CORES = 8
BL = B // N_CORES          # 1024 rows per core
P = 128                    # partitions
NB = BL // P               # 8 batch chunks per core
ND = D // P                # 8 d chunks
NCC = C // P               # 8 class chunks (partition-major)
NBF = BL // 512            # 2 batch free-dim chunks
NCF = C // 512             # 2 class free-dim chunks
NBLK = NCC * NBF           # 16 sim blocks
EPS = 1e-12

F32 = mybir.dt.float32
BF16 = mybir.dt.bfloat16
AF = mybir.ActivationFunctionType
ALU = mybir.AluOpType
AX = mybir.AxisListType

_NC_CACHE = {}

def build_nc():
    if "nc" in _NC_CACHE:
        return _NC_CACHE["nc"]
    import concourse.bass_isa as bass_isa

    nc = bacc.Bacc(
        "TRN2", target_bir_lowering=False, debug=False, num_devices=N_CORES
    )
    emb_i = nc.dram_tensor("emb_i", [BL, D], F32, kind="ExternalInput")
    emb_jT = nc.dram_tensor("emb_jT", [D, BL], F32, kind="ExternalInput")
    labels_colmat = nc.dram_tensor("labels_colmat", [P, NB], F32, kind="ExternalInput")
    label_bcast = nc.dram_tensor("label_bcast", [P, BL], F32, kind="ExternalInput")
    iota_bcast = nc.dram_tensor("iota_bcast", [P, C], F32, kind="ExternalInput")
    ccol = nc.dram_tensor("ccol", [P, NCC], F32, kind="ExternalInput")
    out_partial = nc.dram_tensor("out_partial", [1, 2], F32, kind="ExternalOutput")

    with tile.TileContext(nc) as tc:
        with (
            tc.tile_pool(name="dram", bufs=1, space="DRAM") as dram,
            tc.tile_pool(name="const", bufs=1) as constp,
            tc.tile_pool(name="zjt", bufs=1) as zjtp,
            tc.tile_pool(name="work", bufs=2) as work,
            tc.tile_pool(name="work3", bufs=3) as work3,
            tc.tile_pool(name="dump", bufs=1) as dump,
        ):
            cc_in = dram.tile([D + 1, C], BF16)
            cc_out = dram.tile([D + 1, C], BF16, addr_space="Shared")

            ones_col = constp.tile([P, 1], F32)
            nc.vector.memset(ones_col[:], 1.0)
            ones_bf = constp.tile([P, 1], BF16)
            nc.vector.memset(ones_bf[:], 1.0)
            two_col = constp.tile([P, 1], F32)
            nc.vector.memset(two_col[:], 2.0)
            lab_cm = constp.tile([P, NB], F32)
            nc.sync.dma_start(lab_cm[:], labels_colmat[:])
            lab_bc = constp.tile([P, BL], F32)
            nc.sync.dma_start(lab_bc[:], label_bcast[:])
            iota_bc = constp.tile([P, C], F32)
            nc.sync.dma_start(iota_bc[:], iota_bcast[:])
            ccol_t = constp.tile([P, NCC], F32)
            nc.sync.dma_start(ccol_t[:], ccol[:])

            # ---------------- phase A ----------------
            with (
                tc.tile_pool(name="phA", bufs=1) as pa,
                tc.tile_pool(name="psA", bufs=2, space="PSUM") as psA,
            ):
                z_i = [pa.tile([P, D], BF16, name=f"zi{b}") for b in range(NB)]
                oh = [pa.tile([P, C], BF16, name=f"oh{b}") for b in range(NB)]
                sq_dump = dump.tile([P, D], F32, name="sq_dump")
                # per-chunk pipelined norms; Square/Sqrt share one ACT table
                e_last = None
                for b in range(NB):
                    e = work.tile([P, D], F32, tag="embi")
                    nc.sync.dma_start(e[:], emb_i[b * P : (b + 1) * P, :])
                    e_last = e
                    ss = work.tile([P, 1], F32, tag="ss")
                    nc.scalar.activation(sq_dump[:], e[:], AF.Square, accum_out=ss[:])
                    nrm = work.tile([P, 1], F32, tag="nrm")
                    nc.scalar.activation(nrm[:], ss[:], AF.Sqrt)
                    nc.vector.tensor_scalar(nrm[:], nrm[:], EPS, None, ALU.max)
                    inv = work.tile([P, 1], F32, tag="inv")
                    nc.vector.reciprocal(inv[:], nrm[:])
                    nc.vector.tensor_scalar(z_i[b][:], e[:], inv[:], None, ALU.mult)
                    nc.vector.tensor_scalar(
                        oh[b][:], iota_bc[:], lab_cm[:, b : b + 1], None, ALU.is_equal
                    )

                # segT matmuls: out[d_chunk, c] = sum_b z_i[b, d] * onehot[b, c]
                for d in range(ND):
                    for cf in range(NCF):
                        ps = psA.tile([P, 512], F32, tag="seg")
                        for b in range(NB):
                            nc.tensor.matmul(
                                ps[:],
                                z_i[b][:, d * P : (d + 1) * P],
                                oh[b][:, cf * 512 : (cf + 1) * 512],
                                start=(b == 0),
                                stop=(b == NB - 1),
                            )
                        so = work3.tile([P, 512], BF16, tag="segout")
                        nc.scalar.copy(so[:], ps[:])
                        nc.sync.dma_start(
                            cc_in[d * P : (d + 1) * P, cf * 512 : (cf + 1) * 512],
                            so[:],
                        )

                # counts as a [1, C] row: cnt[c] = sum_b onehot[b, c]
                cnt_ps = psA.tile([1, C], F32, tag="cntrow")
                for half in range(NCF):
                    for b in range(NB):
                        nc.tensor.matmul(
                            cnt_ps[:, half * 512 : (half + 1) * 512],
                            ones_bf[:],
                            oh[b][:, half * 512 : (half + 1) * 512],
                            start=(b == 0),
                            stop=(b == NB - 1),
                        )
                cnt_row = work.tile([1, C], BF16, tag="cntrowsb")
                nc.scalar.copy(cnt_row[:], cnt_ps[:])
                nc.sync.dma_start(cc_in[D : D + 1, :], cnt_row[:])

            nc.gpsimd.collective_compute(
                "AllReduce",
                ALU.add,
                replica_groups=[list(range(N_CORES))],
                ins=[cc_in[:].opt()],
                outs=[cc_out[:].opt()],
            )

            # ---------------- phase B (overlaps collective) ----------------
            zjt = [zjtp.tile([P, BL], BF16, name=f"zjt{d}") for d in range(ND)]
            with tc.tile_pool(name="embt", bufs=1) as embtp:
                embT = [embtp.tile([P, BL], F32, name=f"embT{d}") for d in range(ND)]
                acc = embtp.tile([P, BL], F32, name="acc")
                for d in range(ND):
                    # gate emb_jT transfers behind the last emb_i load so
                    # phase A input DMAs get the full HBM bandwidth first
                    nc.vector.tensor_copy(embT[d][0:1, 0:1], e_last[0:1, 0:1])
                    nc.sync.dma_start(embT[d][:], emb_jT[d * P : (d + 1) * P, :])
                    sq2 = work.tile([P, BL], F32, tag="sqscr2")
                    nc.scalar.activation(sq2[:], embT[d][:], AF.Square)
                    if d == 0:
                        nc.vector.tensor_copy(acc[:], sq2[:])
                    else:
                        nc.vector.tensor_add(acc[:], acc[:], sq2[:])
                nrm2 = embtp.tile([P, BL], F32, name="nrm2")
                nc.gpsimd.partition_all_reduce(
                    nrm2[:], acc[:], channels=P, reduce_op=bass_isa.ReduceOp.add
                )
                nc.scalar.activation(nrm2[:], nrm2[:], AF.Sqrt)
                nc.vector.tensor_scalar(nrm2[:], nrm2[:], EPS, None, ALU.max)
                invb = embtp.tile([P, BL], F32, name="invb")
                nc.vector.reciprocal(invb[:], nrm2[:])
                for d in range(ND):
                    nc.vector.tensor_tensor(zjt[d][:], embT[d][:], invb[:], ALU.mult)

            # ---------------- phase C ----------------
            with (
                tc.tile_pool(name="phC", bufs=1) as pcpool,
                tc.tile_pool(name="psC", bufs=2, space="PSUM") as psC,
                tc.tile_pool(name="psFin", bufs=1, space="PSUM") as psFin,
                tc.tile_pool(name="psSim", bufs=4, space="PSUM") as psSim,
            ):
                segT = [pcpool.tile([P, C], BF16, name=f"segT{d}") for d in range(ND)]
                # ssq accumulated on DVE as segT chunks arrive
                ssqacc = pcpool.tile([P, C], F32, name="ssqacc")
                for d in range(ND):
                    nc.sync.dma_start(segT[d][:], cc_out[d * P : (d + 1) * P, :])
                    sqc = work.tile([P, C], F32, tag="sqc")
                    nc.scalar.activation(sqc[:], segT[d][:], AF.Square)
                    if d == 0:
                        nc.vector.tensor_copy(ssqacc[:], sqc[:])
                    else:
                        nc.vector.tensor_add(ssqacc[:], ssqacc[:], sqc[:])
                ssqred = pcpool.tile([P, C], F32, name="ssqred")
                nc.gpsimd.partition_all_reduce(
                    ssqred[:], ssqacc[:], channels=P, reduce_op=bass_isa.ReduceOp.add
                )
                cnt_rowb = constp.tile([1, C], BF16)
                nc.sync.dma_start(cnt_rowb[:], cc_out[D : D + 1, :])
                cnt_row2 = constp.tile([1, C], F32)
                nc.vector.tensor_copy(cnt_row2[:], cnt_rowb[:])
                ident1 = constp.tile([1, 1], F32)
                nc.vector.memset(ident1[:], 1.0)
                cnt_col = constp.tile([P, NCC], F32)
                ssq_col = constp.tile([P, NCC], F32)
                for cc in range(NCC):
                    pt = psC.tile([P, 1], F32, tag="col1")
                    nc.tensor.transpose(
                        pt[:], cnt_row2[0:1, cc * P : (cc + 1) * P], ident1[:]
                    )
                    nc.vector.tensor_copy(cnt_col[:, cc : cc + 1], pt[:])
                    pq = psC.tile([P, 1], F32, tag="col1")
                    nc.tensor.transpose(
                        pq[:], ssqred[0:1, cc * P : (cc + 1) * P], ident1[:]
                    )
                    nc.vector.tensor_copy(ssq_col[:, cc : cc + 1], pq[:])
                ic = constp.tile([P, NCC], F32)
                nc.vector.reciprocal(ic[:], cnt_col[:])
                scale_col = constp.tile([P, NCC], F32)
                nc.vector.tensor_scalar(scale_col[:], ic[:], -2.0, None, ALU.mult)
                ic2 = constp.tile([P, NCC], F32)
                nc.vector.tensor_tensor(ic2[:], ic[:], ic[:], ALU.mult)
                bias_col = constp.tile([P, NCC], F32)
                nc.vector.tensor_tensor(bias_col[:], ssq_col[:], ic2[:], ALU.mult)
                nc.vector.tensor_scalar(bias_col[:], bias_col[:], 1.0, None, ALU.add)

                sp_st = constp.tile([P, NBLK], F32)
                dg_st = constp.tile([P, NBLK], F32)
                sp_dump = dump.tile([P, 512], F32, name="sp_dump")
                with tc.tile_pool(name="rall", bufs=1) as rallp:
                    r_all = [
                        rallp.tile([P, 512], F32, name=f"r{blk}")
                        for blk in range(NBLK)
                    ]
                    # pass 1: matmuls + Sqrt per block (single ACT table),
                    # DVE diag term per block
                    for cc in range(NCC):
                        for bf in range(NBF):
                            blk = cc * NBF + bf
                            ps = psSim.tile([P, 512], F32, tag="sim")
                            for d in range(ND):
                                nc.tensor.matmul(
                                    ps[:],
                                    segT[d][:, cc * P : (cc + 1) * P],
                                    zjt[d][:, bf * 512 : (bf + 1) * 512],
                                    start=(d == 0),
                                    stop=(d == ND - 1),
                                )
                            nc.scalar.activation(
                                r_all[blk][:],
                                ps[:],
                                AF.Sqrt,
                                bias=bias_col[:, cc : cc + 1],
                                scale=scale_col[:, cc : cc + 1],
                            )
                            # diag term in one fused DVE op:
                            # (label == c) * r, accumulated along b
                            prod = work.tile([P, 512], F32, tag="prod")
                            nc.vector.scalar_tensor_tensor(
                                prod[:],
                                lab_bc[:, bf * 512 : (bf + 1) * 512],
                                ccol_t[:, cc : cc + 1],
                                r_all[blk][:],
                                op0=ALU.is_equal,
                                op1=ALU.mult,
                                accum_out=dg_st[:, blk : blk + 1],
                            )
                    # pass 2: Exp + Ln batched (one table switch total).
                    # two_gate copies two_col after the last Sqrt so every Exp
                    # data-depends on all Sqrts -> scheduler cannot interleave
                    # Exp/Ln into the Sqrt stretch (would thrash ACT tables).
                    two_gate = constp.tile([P, 1], F32)
                    gate_probe = work.tile([P, 1], F32, tag="gateprobe")
                    nc.vector.tensor_reduce(
                        gate_probe[:], r_all[NBLK - 1][:, 0:2], axis=AX.X, op=ALU.max
                    )
                    nc.vector.tensor_scalar(
                        two_gate[:], gate_probe[:], 0.0, 2.0, ALU.mult, ALU.add
                    )
                    ex_all = [
                        rallp.tile([P, 512], F32, name=f"ex{blk}")
                        for blk in range(NBLK)
                    ]
                    for blk in range(NBLK):
                        nc.scalar.activation(
                            ex_all[blk][:],
                            r_all[blk][:],
                            AF.Exp,
                            bias=two_gate[:],
                            scale=-1.0,
                        )
                    one_gate = constp.tile([P, 1], F32)
                    gate_probe2 = work.tile([P, 1], F32, tag="gateprobe")
                    nc.vector.tensor_reduce(
                        gate_probe2[:],
                        ex_all[NBLK - 1][:, 0:2],
                        axis=AX.X,
                        op=ALU.max,
                    )
                    nc.vector.tensor_scalar(
                        one_gate[:], gate_probe2[:], 0.0, 1.0, ALU.mult, ALU.add
                    )
                    for blk in range(NBLK):
                        nc.scalar.activation(
                            sp_dump[:],
                            ex_all[blk][:],
                            AF.Ln,
                            bias=one_gate[:],
                            accum_out=sp_st[:, blk : blk + 1],
                        )

                # final on-device reduction to two scalars
                pf = psFin.tile([1, NBLK], F32, tag="fin")
                nc.tensor.matmul(pf[:], ones_col[:], sp_st[:], start=True, stop=True)
                sp_row = constp.tile([1, NBLK], F32)
                nc.vector.tensor_copy(sp_row[:], pf[:])
                sp_tot = constp.tile([1, 1], F32)
                nc.vector.tensor_reduce(sp_tot[:], sp_row[:], axis=AX.X, op=ALU.add)

                pf2 = psFin.tile([1, NBLK], F32, tag="fin")
                nc.tensor.matmul(pf2[:], ones_col[:], dg_st[:], start=True, stop=True)
                dg_row = constp.tile([1, NBLK], F32)
                nc.vector.tensor_copy(dg_row[:], pf2[:])
                dg_tot = constp.tile([1, 1], F32)
                nc.vector.tensor_reduce(dg_tot[:], dg_row[:], axis=AX.X, op=ALU.add)

                nc.sync.dma_start(out_partial[0:1, 0:1], sp_tot[:])
                nc.sync.dma_start(out_partial[0:1, 1:2], dg_tot[:])

    nc.compile()
    _NC_CACHE["nc"] = nc
    return nc


def make_in_maps(emb_i, emb_j, labels):
    emb_i = np.ascontiguousarray(np.asarray(emb_i, dtype=np.float32))
    emb_j = np.ascontiguousarray(np.asarray(emb_j, dtype=np.float32))
    labf = np.asarray(labels).astype(np.float32)
    iota_bc = np.ascontiguousarray(
        np.broadcast_to(np.arange(C, dtype=np.float32)[None, :], (P, C))
    )
    ccol = np.ascontiguousarray(
        np.arange(P, dtype=np.float32)[:, None]
        + P * np.arange(NCC, dtype=np.float32)[None, :]
    )
    in_maps = []
    for k in range(N_CORES):
        sl = slice(k * BL, (k + 1) * BL)
        lab_k = labf[sl]
        in_maps.append(
            {
                "emb_i": emb_i[sl],
                "emb_jT": np.ascontiguousarray(emb_j[sl].T),
                "labels_colmat": np.ascontiguousarray(lab_k.reshape(NB, P).T),
                "label_bcast": np.ascontiguousarray(
                    np.broadcast_to(lab_k[None, :], (P, BL))
                ),
                "iota_bcast": iota_bc,
                "ccol": ccol,
            }
        )
    return in_maps


def combine_partials(results):
    tot = 0.0
    for k in range(N_CORES):
        p = np.asarray(results[k]["out_partial"], dtype=np.float64)
        tot += p[0, 0] + p[0, 1]
    loss = (tot - 2.0 * B) / (B * C)
    return np.asarray(np.float32(loss))


def run(emb_i, emb_j, labels, **run_kwargs):
    nc = build_nc()
    in_maps = make_in_maps(emb_i, emb_j, labels)
    res = bass_utils.run_bass_kernel_spmd(
        nc, in_maps, core_ids=list(range(N_CORES)), **run_kwargs
    )
    return combine_partials(res.results), res


def kernel(emb_i, emb_j, labels):
    loss, _ = run(emb_i, emb_j, labels)
    return loss
